# revision 51
# baseline (speedup 1.0000x reference)
"""Trainium2 Bass kernel for a batched GAT layer (BGATLayer).

Reference computation (per batch b of B=16, N=1024 nodes, F=512 features):
    h   = x @ W                                   # [N, F]
    s1  = h @ a1 ; s2 = h @ a2                    # [N]
    e   = leakyrelu(s1[:,None] + s2[None,:], 0.2) # [N, N]
    att = softmax(e, axis=1)                      # row softmax
    out = elu(att @ h + beta * h)                 # [N, F]

Sharding: batch B=16 split across 8 NeuronCores (2 batches/core, data
parallel); W/a/beta replicated.

Design notes (v1 baseline measured 147us; this version ~105us):
  * All matmul operands are bf16 (tolerance is 2e-2; bf16 lands ~5e-3).
    Streaming rate equals f32r@512 (1 cyc/row) but LDWEIGHTS halves and
    SBUF shrinks.  Output is bf16 too (host casts back to fp32).
  * The attention exponential is FACTORIZED so no NxN ACT pass exists:
      u = exp(lrelu(s1+s2)) = max(e^{s1}e^{s2}, e^{.2s1}e^{.2s2})
    exps run on s VECTORS only (ACT rows/[128,8] cols, ~1us total);
    e^{s1} rows become [128,N] tiles via K=1 ones-outer-product PE
    matmuls (partition_broadcast DMA measured 12.5us -- never use it
    for wide tiles); each uT tile is then 3 fast-mode DVE ops
    (tensor_scalar 4x_2p with per-partition scalar E2 columns +
    tensor_tensor max 2x_1p).  scalar_tensor_tensor has NO fast modes;
    tensor_scalar/tensor_copy do (2-byte packed -> 2x, +SBUF-only ->
    4x) -- this dictates the op selection throughout.
  * T and B phases interleave per-tile so the PSUM->SBUF drain copies
    (ACT) never pace the PE; weight DMAs ride the scalar engine's
    hardware DMA queue so the 4MB x stream can't starve them.
  * rowsum via ones-stationary matmuls; reciprocal through the DRAM
    row->column roundtrip (compute engines can't cross partitions).
  * epilogue per tile: v1 = p*recip (ACT scale-copy, per-partition AP
    scale), v = v1+h (DVE 2x), m = min(v,0) (DVE 4x), em = exp (ACT),
    out = max(em-1, v) (DVE), trailing the p matmuls tile-by-tile.
  * gpsimd tensor ops break neuronx compilation here -- DVE/ACT only.
  * Remaining limiter: PE busy ~78us with HAM k=4/8 duty-cycle windows
    (~7us every ~20us even under continuous load) + ~9us startup.
"""

import sys

sys.path.insert(0, "/opt/trn_rl_repo")

from contextlib import ExitStack

import numpy as np

import concourse.bacc as bacc
import concourse.bass as bass
import concourse.mybir as mybir
from concourse.bass_utils import run_bass_kernel_spmd
from concourse.masks import make_identity
from concourse.tile import TileContext

P = 128
N_NODES = 1024
F = 512
B_TOTAL = 16
N_CORES = 8
B_PER_CORE = B_TOTAL // N_CORES
NK = F // P  # 4 contraction chunks for x @ W
NN = N_NODES // P  # 8 node chunks
ALPHA = 0.2

F32 = mybir.dt.float32
F32R = mybir.dt.float32r
BF16 = mybir.dt.bfloat16
AL = mybir.AluOpType
AF = mybir.ActivationFunctionType


def _r(ap):
    """float32r view of an fp32 AP (PE reduced-precision matmul mode)."""
    return ap.bitcast(F32R)


def build_nc(beta_val: float = 1.0) -> bass.Bass:
    nc = bacc.Bacc("TRN2")
    x_d = nc.dram_tensor("x", [B_PER_CORE, N_NODES, F], F32, kind="ExternalInput")
    w_d = nc.dram_tensor("W", [F, F], F32, kind="ExternalInput")
    a_d = nc.dram_tensor("a", [2 * F, 1], F32, kind="ExternalInput")
    beta_d = nc.dram_tensor("beta", [1], F32, kind="ExternalInput")
    out_d = nc.dram_tensor("out", [B_PER_CORE, N_NODES, F], BF16, kind="ExternalOutput")
    # scratch for row->per-partition-column roundtrips
    r_d = nc.dram_tensor("r_scratch", [B_PER_CORE, N_NODES], F32)
    s_d = nc.dram_tensor("s_scratch", [B_PER_CORE, N_NODES], F32)

    with TileContext(nc) as tc, ExitStack() as ctx:
        # ---------------- pools ----------------
        singles = ctx.enter_context(tc.tile_pool(name="singles", bufs=1))
        xin = ctx.enter_context(tc.tile_pool(name="xin", bufs=16))
        xtp = ctx.enter_context(tc.tile_pool(name="xtp", bufs=2))  # xT bf16
        hpool = ctx.enter_context(tc.tile_pool(name="hpool", bufs=16))
        spool = ctx.enter_context(tc.tile_pool(name="spool", bufs=2))
        utp = ctx.enter_context(tc.tile_pool(name="utp", bufs=16))
        tpool = ctx.enter_context(tc.tile_pool(name="tpool", bufs=3))
        qp = ctx.enter_context(tc.tile_pool(name="qp", bufs=2))
        epool = ctx.enter_context(tc.tile_pool(name="epool", bufs=4))
        # PSUM: ps_tr 2x[128,512](2 banks) ps_mm 3x[128,512](3) ps_s [2,1024](2)
        # ps_e 1x[128,512](1) -> 8 banks
        ps_tr = ctx.enter_context(tc.tile_pool(name="ps_tr", bufs=2, space="PSUM"))
        ps_mm = ctx.enter_context(tc.tile_pool(name="ps_mm", bufs=3, space="PSUM"))
        ps_s = ctx.enter_context(tc.tile_pool(name="ps_s", bufs=1, space="PSUM"))
        ps_e = ctx.enter_context(tc.tile_pool(name="ps_e", bufs=1, space="PSUM"))

        # ---------------- prologue ----------------
        identf = singles.tile([P, P], F32, tag="identf")
        make_identity(nc, identf)
        ident = singles.tile([P, P], F32, tag="ident")
        nc.scalar.copy(out=_r(ident), in_=identf)

        ones2b = singles.tile([P, 2], BF16, tag="ones2b")
        nc.gpsimd.memset(ones2b, 1.0)
        # [1,128] ones row, f32r-written: stationary for the K=1 E-broadcast
        onesrowf = singles.tile([1, P], F32, tag="onesrowf")
        nc.gpsimd.memset(onesrowf, 1.0)
        onesrow = singles.tile([1, P], F32, tag="onesrow")
        nc.scalar.copy(out=_r(onesrow), in_=onesrowf)

        a_flat = a_d.rearrange("f one -> (f one)")
        a1b = singles.tile([P, F], F32, tag="a1b")
        a2b = singles.tile([P, F], F32, tag="a2b")
        beta_sb = singles.tile([1, 1], F32, tag="beta_sb")
        w_sb = []
        wb = []
        for k in range(NK):
            wk = singles.tile([P, F], F32, tag=f"w_sb{k}")
            w_sb.append(wk)
            wbk = singles.tile([P, F], BF16, tag=f"wb{k}")
            wb.append(wbk)
        w12b = singles.tile([P, 2 * NK], BF16, tag="w12b")

        def load_weights():
            # weight DMAs ride the scalar engine's hardware queue so they
            # don't serialize behind the 4MB x stream on the sync queue
            nc.scalar.dma_start(out=a1b, in_=a_flat[0:F].partition_broadcast(P))
            nc.scalar.dma_start(out=a2b, in_=a_flat[F : 2 * F].partition_broadcast(P))
            # beta lands in SBUF only to keep the input bound (value baked)
            nc.scalar.dma_start(out=beta_sb, in_=beta_d[0:1].unsqueeze(0))
            for k in range(NK):
                nc.scalar.dma_start(out=w_sb[k], in_=w_d[k * P : (k + 1) * P, :])
                # bf16 copy of W for the h matmul (moving operand); DVE so
                # the ACT queue stays clear for the xt0 drain copies
                nc.vector.tensor_copy(out=wb[k], in_=w_sb[k])
                w12f = qp.tile([P, 2], F32, tag="w12f")
                prod = qp.tile([P, F], F32, tag="wa_prod")
                for j, ab in enumerate((a1b, a2b)):
                    # W@a via elementwise mult + per-partition accumulator
                    nc.vector.scalar_tensor_tensor(
                        out=prod, in0=w_sb[k], scalar=1.0, in1=ab,
                        op0=AL.mult, op1=AL.mult,
                        accum_out=w12f[:, j : j + 1],
                    )
                nc.vector.tensor_copy(out=w12b[:, 2 * k : 2 * k + 2], in_=w12f)

        # ---------------- PE warm-up ----------------
        # hold the activity monitor busy during the initial DMA window so
        # real matmuls start at the max clock
        for _ in range(8):
            wp = ps_tr.tile([P, F], F32, tag="ps_tr")
            nc.tensor.transpose(_r(wp[:, 0:P]), _r(ident), _r(ident))
            nc.tensor.transpose(_r(wp[:, P : 2 * P]), _r(ident), _r(ident))

        # ---------------- per-batch state ----------------
        xt_alls = {}
        h_sbs = {}
        uts = {}
        rcols = {}
        e1bs = {}
        e1abs = {}
        e2cols = {}
        e2acols = {}
        x_tiles = {}

        def phase_A_dma(b):  # issue all x loads for this batch
            x_tiles[b] = []
            for n in range(NN):
                x_t = xin.tile([P, F], F32, tag="x_t")
                nc.sync.dma_start(out=_r(x_t), in_=_r(x_d[b, n * P : (n + 1) * P, :]))
                x_tiles[b].append(x_t)

        def emit_T_tile(b, n):
            xt_all = xt_alls[b]
            x_t = x_tiles[b][n]
            xp = ps_tr.tile([P, F], F32, tag="ps_tr")
            for k in range(NK):
                nc.tensor.transpose(
                    _r(xp[:, k * P : (k + 1) * P]),
                    _r(x_t[:, k * P : (k + 1) * P]),
                    _r(ident),
                )
            dst = xt_all.rearrange("p (k c) -> p k c", k=NK)[
                :, :, n * P : (n + 1) * P
            ]
            src = xp.rearrange("p (k c) -> p k c", k=NK)
            nc.scalar.copy(out=dst, in_=src)

        def emit_B_tile(b, n):
            xt_all = xt_alls[b]
            h_ps = ps_mm.tile([P, F], F32, tag="ps_mm")
            for k in range(NK):
                nc.tensor.matmul(
                    h_ps,
                    lhsT=xt_all[:, k * N_NODES + n * P : k * N_NODES + (n + 1) * P],
                    rhs=wb[k],
                    start=(k == 0),
                    stop=(k == NK - 1),
                )
            ht = hpool.tile([P, F], BF16, tag="h_sb")
            nc.scalar.copy(out=ht, in_=h_ps)
            h_sbs[b].append(ht)

        def phase_TB(b):
            # per-tile interleave: transposes for tile n overlap the h
            # matmuls for tile n-1, so the PSUM drain copies (ACT) never
            # pace the PE -- each tile gives ACT ~1.3us of copies against
            # ~2.9us of PE work
            xt_all = xtp.tile([P, NK * N_NODES], BF16, tag="xt_all")
            xt_alls[b] = xt_all
            h_sbs[b] = []
            for n in range(NN):
                emit_T_tile(b, n)
                if n > 0:
                    emit_B_tile(b, n - 1)
            emit_B_tile(b, NN - 1)

        e1rows = {}
        e1arows = {}

        def phase_S(b):
            # s rows; u's factorization u = max(e^{s1}e^{s2},
            # e^{.2 s1}e^{.2 s2}) needs exps only on the s VECTORS, not on
            # the NxN matrix: E2/E2a as [128,8] columns (bias scalars),
            # E1/E1a as [1,N] rows that a K=1 PE matmul broadcasts to
            # [128,N] (a partition_broadcast DMA measured 12.5us; the PE
            # outer product is ~0.5us).
            xt_all = xt_alls[b]
            s_ps = ps_s.tile([2, N_NODES], F32, tag="ps_s")
            for k in range(NK):
                for hh in range(2):
                    nc.tensor.matmul(
                        s_ps[:, hh * F : (hh + 1) * F],
                        lhsT=w12b[:, 2 * k : 2 * k + 2],
                        rhs=xt_all[:, k * N_NODES + hh * F : k * N_NODES + (hh + 1) * F],
                        start=(k == 0),
                        stop=(k == NK - 1),
                    )
            # E1/E1a rows straight from PSUM (f32r-written: they feed the
            # broadcast matmul) -- no copy on the critical path
            e1row = spool.tile([1, N_NODES], F32, tag="e1row")
            e1rows[b] = e1row
            nc.scalar.activation(out=_r(e1row), in_=s_ps[0:1, :], func=AF.Exp)
            e1arow = spool.tile([1, N_NODES], F32, tag="e1arow")
            e1arows[b] = e1arow
            nc.scalar.activation(
                out=_r(e1arow), in_=s_ps[0:1, :], func=AF.Exp, scale=ALPHA
            )
            s_sb = spool.tile([2, N_NODES], F32, tag="s_sb")
            nc.vector.tensor_copy(out=s_sb, in_=s_ps)
            # s2 row -> per-partition columns through DRAM, then tiny exps
            nc.sync.dma_start(out=s_d[b].unsqueeze(0), in_=s_sb[1:2, :])
            s2col = spool.tile([P, NN], F32, tag="s2col")
            nc.sync.dma_start(out=s2col, in_=s_d[b].rearrange("(n p) -> p n", p=P))
            e2col = spool.tile([P, NN], F32, tag="e2col")
            nc.scalar.activation(out=e2col, in_=s2col, func=AF.Exp)
            e2cols[b] = e2col
            e2acol = spool.tile([P, NN], F32, tag="e2acol")
            nc.scalar.activation(out=e2acol, in_=s2col, func=AF.Exp, scale=ALPHA)
            e2acols[b] = e2acol

        def emit_E_bcast(b):
            # e1b/e1ab[p, i] = E1/E1a[i] via ones-column outer product;
            # bf16 SBUF copies so the C-phase tensor_scalars hit 4x_2p
            e1b = spool.tile([P, N_NODES], BF16, tag="e1b")
            e1bs[b] = e1b
            e1ab = spool.tile([P, N_NODES], BF16, tag="e1ab")
            e1abs[b] = e1ab
            for row, dstf in ((e1rows[b], e1b), (e1arows[b], e1ab)):
                for hh in range(2):
                    bp = ps_e.tile([P, F], F32, tag="ps_e")
                    nc.tensor.matmul(
                        bp,
                        lhsT=_r(onesrow),
                        rhs=_r(row[0:1, hh * F : (hh + 1) * F]),
                        start=True,
                        stop=True,
                    )
                    nc.scalar.copy(out=dstf[:, hh * F : (hh + 1) * F], in_=bp)

        def emit_C_tile(b, j):
            # uT[j][p, i] = max(E1[i]E2[jp], E1a[i]E2a[jp]) -- 3 DVE ops in
            # 2x fast mode (SBUF-only operands; bf16 outs make the max
            # 2x_1p-eligible; per-partition scalars are dtype-exempt).
            # Zero ACT, zero PE.
            t1 = tpool.tile([P, N_NODES], BF16, tag="t1")
            nc.vector.tensor_scalar(
                out=t1, in0=e1bs[b], scalar1=e2cols[b][:, j : j + 1], scalar2=None,
                op0=AL.mult,
            )
            t2 = tpool.tile([P, N_NODES], BF16, tag="t2")
            nc.vector.tensor_scalar(
                out=t2, in0=e1abs[b], scalar1=e2acols[b][:, j : j + 1], scalar2=None,
                op0=AL.mult,
            )
            u = utp.tile([P, N_NODES], BF16, tag="ut")
            nc.vector.tensor_tensor(out=u, in0=t1, in1=t2, op=AL.max)
            uts[b][j] = u

        def phase_C(b):
            uts[b] = [None] * NN
            for j in range(NN):
                emit_C_tile(b, j)


        def phase_R(b):  # rowsum -> reciprocal columns
            ut = uts[b]
            rs_ps = ps_s.tile([2, N_NODES], F32, tag="ps_s")
            for j in range(NN):
                for hh in range(2):
                    nc.tensor.matmul(
                        rs_ps[:, hh * F : (hh + 1) * F],
                        lhsT=ones2b,
                        rhs=ut[j][:, hh * F : (hh + 1) * F],
                        start=(j == 0),
                        stop=(j == NN - 1),
                    )
            rrow = spool.tile([1, N_NODES], F32, tag="rrow")
            nc.vector.tensor_copy(out=rrow, in_=rs_ps[0:1, :])
            nc.sync.dma_start(out=r_d[b].unsqueeze(0), in_=rrow)
            rcraw = spool.tile([P, NN], F32, tag="rcraw")
            nc.sync.dma_start(out=rcraw, in_=r_d[b].rearrange("(n p) -> p n", p=P))
            rcol = spool.tile([P, NN], F32, tag="rcol")
            rcols[b] = rcol
            nc.vector.reciprocal(out=rcol, in_=rcraw)

        def emit_DE_tile(b, n):  # p[n] = u @ h + fused ELU epilogue
            ut, h_sb, rcol = uts[b], h_sbs[b], rcols[b]
            p_ps = ps_mm.tile([P, F], F32, tag="ps_mm")
            for j in range(NN):
                nc.tensor.matmul(
                    p_ps,
                    lhsT=ut[j][:, n * P : (n + 1) * P],
                    rhs=h_sb[j],
                    start=(j == 0),
                    stop=(j == NN - 1),
                )
            hin = h_sb[n]
            if beta_val != 1.0:
                hb = epool.tile([P, F], BF16, tag="hb")
                nc.vector.tensor_scalar_mul(hb, hin, float(beta_val))
                hin = hb
            # v = p*(1/rowsum) + beta*h: ACT scale-copy + DVE bf16 add;
            # elu via relu-trick (ACT) + bf16 fast-mode DVE ops
            v1 = epool.tile([P, F], BF16, tag="v1")
            nc.scalar.activation(
                out=v1, in_=p_ps, func=AF.Copy, scale=rcol[:, n : n + 1]
            )
            v = epool.tile([P, F], BF16, tag="v")
            nc.vector.tensor_tensor(out=v, in0=v1, in1=hin, op=AL.add)
            m = epool.tile([P, F], BF16, tag="m")
            nc.vector.tensor_scalar(
                out=m, in0=v, scalar1=0.0, scalar2=None, op0=AL.min
            )
            em = epool.tile([P, F], BF16, tag="em")
            nc.scalar.activation(out=em, in_=m, func=AF.Exp)
            o1 = epool.tile([P, F], BF16, tag="o1")
            nc.vector.tensor_scalar(
                out=o1, in0=em, scalar1=-1.0, scalar2=None, op0=AL.add
            )
            o = epool.tile([P, F], BF16, tag="o")
            nc.vector.tensor_tensor(out=o, in0=o1, in1=v, op=AL.max)
            nc.sync.dma_start(out=out_d[b, n * P : (n + 1) * P, :], in_=o)

        # ------------- software-pipelined emission -------------
        # PE order: warmup T0 S0 B0 T1 S1 B1 R0 DE0 R1 DE1 -- back-to-back
        # matmuls, never paced by ACT/DVE.  C phases are pure ACT/DVE and
        # run concurrently (C0 under B0/T1, C1 under B1/DE0).
        phase_A_dma(0)
        load_weights()
        phase_TB(0)
        phase_S(0)
        emit_E_bcast(0)
        phase_A_dma(1)
        phase_C(0)
        phase_TB(1)
        phase_S(1)
        phase_R(0)
        emit_E_bcast(1)
        # C1 fully front-loaded: DVE has slack and R1/DE1 must never wait
        phase_C(1)
        for n in range(NN):
            if n == 5:
                phase_R(1)
            emit_DE_tile(0, n)
        for n in range(NN):
            emit_DE_tile(1, n)

    nc.finalize()
    return nc


_NC_CACHE = {}


def _get_nc(beta_val: float) -> bass.Bass:
    key = float(beta_val)
    if key not in _NC_CACHE:
        _NC_CACHE[key] = build_nc(beta_val=key)
    return _NC_CACHE[key]


def kernel(x, W, a, beta, _trace=False, _mm_fp32=False):
    x = np.ascontiguousarray(x, dtype=np.float32)
    W = np.ascontiguousarray(W, dtype=np.float32)
    a = np.ascontiguousarray(a, dtype=np.float32)
    beta = np.ascontiguousarray(beta, dtype=np.float32)

    nc = _get_nc(float(beta.reshape(-1)[0]))
    in_maps = [
        {
            "x": x[c * B_PER_CORE : (c + 1) * B_PER_CORE],
            "W": W,
            "a": a,
            "beta": beta,
        }
        for c in range(N_CORES)
    ]
    res = run_bass_kernel_spmd(nc, in_maps, core_ids=list(range(N_CORES)), trace=_trace)
    out = np.concatenate(
        [np.asarray(r["out"]).astype(np.float32) for r in res.results], axis=0
    )
    if _trace:
        kernel.last_exec_time_ns = res.exec_time_ns
        kernel.last_results = res
    return out


if __name__ == "__main__":
    rng = np.random.default_rng(0)
    x = rng.standard_normal((B_TOTAL, N_NODES, F), dtype=np.float32)
    W = rng.standard_normal((F, F), dtype=np.float32) * 0.05
    a = rng.standard_normal((2 * F, 1), dtype=np.float32) * 0.05
    beta = np.ones((1,), dtype=np.float32)
    out = kernel(x, W, a, beta)
    print("out", out.shape, out.dtype)
